# revision 1
# baseline (speedup 1.0000x reference)
"""nn_AttentionLayerBlock — 8-core data-parallel kernel for Trainium2.

Sharding: 8 cores = 4 examples x 2 H-halves (64 rows each). Each core
gets its half plus 2 halo rows on each side (zero-padded at image
edges). The two depthwise 3x3 convs shrink the halo by one row each.
The channel-attention Gram matrices (q@k^T, |q|^2, |k|^2 — contracted
over all 16384 pixels) are computed per-half and combined with a psum
over the half axis ("h"); everything else is local to the core.

DIM=192, HEADS=6, HIDDEN=384; x: (4,192,128,128) f32.
"""

import numpy as np
import jax
import jax.numpy as jnp
from jax.sharding import Mesh, PartitionSpec as P
from jax.experimental.shard_map import shard_map
from functools import partial

DIM = 192
HEADS = 6
HC = DIM // HEADS
HIDDEN = int(DIM * 2.0)
EPS = 1e-5
H = W = 128
HALF = 64

_cache = {}


def _ln_c(x, w, b):
    # x: (C, R, W) — layernorm over channel axis per pixel
    mu = jnp.mean(x, axis=0, keepdims=True)
    var = jnp.var(x, axis=0, keepdims=True)
    return (x - mu) / jnp.sqrt(var + EPS) * w[:, None, None] + b[:, None, None]


def _conv1x1(x, w):
    # x: (I, R, W), w: (O, I) -> (O, R, W)
    return jnp.einsum('oi,ihw->ohw', w, x)


def _dw3x3_validH(x, w):
    # x: (C, R, W) -> (C, R-2, W); 'SAME' on W, valid on H
    return jax.lax.conv_general_dilated(
        x[None], w, window_strides=(1, 1), padding=((0, 0), (1, 1)),
        feature_group_count=x.shape[0],
        dimension_numbers=('NCHW', 'OIHW', 'NCHW'))[0]


def _shard_fn(x_sh, mask68, ln3_w, ln3_b, qkv_w, qkv_dw_w, temperature,
              proj_w, ln4_w, ln4_b, pin_w, ffn_dw_w, pout_w):
    # x_sh: (1, 1, C, 68, W) — rows [s-2, e+2) of this core's half,
    # zero-padded outside the image. mask68: (1, 1, 1, 68, 1), 1.0 on
    # real image rows.
    x_sh = x_sh[0, 0]
    mask68 = mask68[0, 0]

    # --- attention branch ---
    y = _ln_c(x_sh, ln3_w, ln3_b) * mask68        # zero the pad rows again
    qkv = _dw3x3_validH(_conv1x1(y, qkv_w), qkv_dw_w)   # (576, 66, W)
    m66 = mask68[:, 1:67]
    qkv = qkv * m66                               # junk/pad rows -> 0
    q, k, v = jnp.split(qkv, 3, axis=0)

    # Gram over OWN rows only (indices 1..64 <-> image rows [s, e))
    qs = q[:, 1:65].reshape(HEADS, HC, HALF * W)
    ks = k[:, 1:65].reshape(HEADS, HC, HALF * W)
    qq = jnp.sum(qs * qs, axis=-1)                # (6, 32)
    kk = jnp.sum(ks * ks, axis=-1)
    qk = jnp.einsum('hcn,hdn->hcd', qs, ks)       # (6, 32, 32)
    qq = jax.lax.psum(qq, 'h')
    kk = jax.lax.psum(kk, 'h')
    qk = jax.lax.psum(qk, 'h')

    rq = 1.0 / jnp.maximum(jnp.sqrt(qq), 1e-12)   # (6, 32)
    rk = 1.0 / jnp.maximum(jnp.sqrt(kk), 1e-12)
    attn = qk * rq[:, :, None] * rk[:, None, :] * temperature
    attn = jax.nn.relu(attn)                      # (6, 32, 32)

    # out = attn @ v on all 66 rows (junk rows are zero)
    vh = v.reshape(HEADS, HC, 66 * W)
    out = jnp.einsum('hcd,hdn->hcn', attn, vh).reshape(DIM, 66, W)
    x2 = _conv1x1(out, proj_w) + x_sh[:, 1:67]    # (192, 66, W)

    # --- GDFN branch ---
    y2 = _ln_c(x2, ln4_w, ln4_b) * m66
    t = _dw3x3_validH(_conv1x1(y2, pin_w), ffn_dw_w)  # (768, 64, W)
    t1, t2 = jnp.split(t, 2, axis=0)
    g = jax.nn.gelu(t1, approximate=False) * t2
    o = _conv1x1(g, pout_w) + x2[:, 1:65]         # (192, 64, W)
    return o[None, None]


def _build():
    if 'fn' in _cache:
        return _cache['fn']
    devices = np.array(jax.devices()[:8]).reshape(4, 2)
    mesh = Mesh(devices, ('b', 'h'))
    wspec = P()
    fn = jax.jit(shard_map(
        _shard_fn, mesh=mesh,
        in_specs=(P('b', 'h'), P('b', 'h')) + (wspec,) * 11,
        out_specs=P('b', 'h'),
        check_rep=False))
    _cache['fn'] = fn
    return fn


def kernel(x, ln3_w, ln3_b, qkv_w, qkv_dw_w, temperature, proj_w,
           ln4_w, ln4_b, pin_w, ffn_dw_w, pout_w):
    x = np.asarray(x, np.float32)
    B = x.shape[0]

    # Host-side sharding: (B, 2, C, 68, W) slabs with 2-row halo.
    xp = np.zeros((B, 2, DIM, 68, W), np.float32)
    mask = np.zeros((B, 2, 1, 68, 1), np.float32)
    for h in range(2):
        s = h * HALF
        lo, hi = s - 2, s + HALF + 2
        clo, chi = max(0, lo), min(H, hi)
        xp[:, h, :, clo - lo:chi - lo] = x[:, :, clo:chi]
        mask[:, h, 0, clo - lo:chi - lo, 0] = 1.0

    fn = _build()
    out = fn(jnp.asarray(xp), jnp.asarray(mask),
             jnp.asarray(ln3_w), jnp.asarray(ln3_b), jnp.asarray(qkv_w),
             jnp.asarray(qkv_dw_w), jnp.asarray(temperature),
             jnp.asarray(proj_w), jnp.asarray(ln4_w), jnp.asarray(ln4_b),
             jnp.asarray(pin_w), jnp.asarray(ffn_dw_w), jnp.asarray(pout_w))
    out = np.asarray(jax.block_until_ready(out))      # (B, 2, C, 64, W)
    return out.transpose(0, 2, 1, 3, 4).reshape(B, DIM, H, W).astype(np.float32)
